# revision 10
# baseline (speedup 1.0000x reference)
"""Trainium2 Bass kernel for the low-rank slot Elman RNN.

Problem:
    per step t:  Wx = x_t @ W_x.T                       [B, D]
                 Uh_s = (U_s @ V_s) h_s   (low-rank slot update)
                 h_s  = tanh(Wx + Uh_s + b)             [B, D] per slot
                 out  = (sum_s C_s h_s) * silu(z_t)     [B, D]
    outputs: out [T, B, D], h [T+1, S, B, D]

Strategy (8 NeuronCores, slot sharding):
  * core s owns slot s's recurrence for the full batch (slot recurrences
    are independent given Wx; only the final combine couples them).
  * Ws = (U_s V_s)^T is merged on-chip once (bf16), so each timestep is a
    single matmul phase: 64 [128x128]x[128x8] weight-stationary MMs.
  * Wx+b is preloaded into PSUM (DVE writes, has_written bits kept set by
    never using start=True in the loop) so the MMs accumulate straight
    onto it and tanh reads PSUM directly -> one cross-engine hop per step.
  * MMs ordered in two d_in blocks and tanh split in halves so each half
    of the new h is ready while the PE still streams the other block.
  * the Wx = x W_x^T + b precompute (2048 N=512 matmuls) is interleaved
    2 MMs per step into the gap where the PE would otherwise wait for
    tanh -- this keeps the tensor engine continuously busy (HAM stays at
    full clock) and makes the precompute free; dummy matmuls keep the
    heater on after Wx work runs out.
  * h and C_s*h stores are batched 8 steps at a time from staging tiles.
  * out-combine: chunked ReduceScatter overlaps the loop; silu(z)*sum is
    applied after the loop (cheap), so only the last chunk's RS is
    exposed.
"""

import numpy as np
import ml_dtypes

# problem dims (hardcoded per contest contract)
D, S, R, T, B = 1024, 8, 256, 1024, 8
P = 128
DC, RC = D // P, R // P  # 8, 2
CB = DC * B              # 64 cols in an h tile: col = dchunk*B + b
H = CB // 2              # half-tile columns
NCORES = 8
WT = 64                  # Wx tile: timesteps per tile (N = WT*B = 512)
G = 8                    # timesteps per store-group
CHUNKS = [128] * 6 + [64] * 4   # ReduceScatter chunking (timesteps)
assert sum(CHUNKS) == T
TSH = sum(c // NCORES for c in CHUNKS)  # out rows per core
PRE_TILES = 2            # Wx tiles computed before the loop

BF16 = ml_dtypes.bfloat16


def _t_indices(core):
    """Timestep indices owned by `core` for the `out` output (RS striping)."""
    idx = []
    off = 0
    for ln in CHUNKS:
        st = ln // NCORES
        idx.extend(range(off + core * st, off + (core + 1) * st))
        off += ln
    return np.array(idx)


def _build():
    import concourse.mybir as mybir
    import concourse.tile as tile
    from concourse import bacc

    f32 = mybir.dt.float32
    bf16 = mybir.dt.bfloat16
    AF = mybir.ActivationFunctionType

    nc = bacc.Bacc("TRN2", target_bir_lowering=False, debug=False,
                   num_devices=NCORES)

    # ---- kernel I/O ----
    xT = nc.dram_tensor("xT", [D, T * B], f32, kind="ExternalInput")
    wxT = nc.dram_tensor("wxT", [D, D], f32, kind="ExternalInput")
    vnat = nc.dram_tensor("vnat", [R, D], bf16, kind="ExternalInput")  # V[s]
    uT = nc.dram_tensor("uT", [R, D], bf16, kind="ExternalInput")      # U[s].T
    bt = nc.dram_tensor("bt", [P, DC], f32, kind="ExternalInput")
    cst = nc.dram_tensor("cst", [P, 1], f32, kind="ExternalInput")
    h0b = nc.dram_tensor("h0b", [P, CB], bf16, kind="ExternalInput")
    zt = nc.dram_tensor("zt", [TSH // G, P, G, CB], f32, kind="ExternalInput")
    hsh = nc.dram_tensor("hsh", [P, T, CB], bf16, kind="ExternalOutput")
    osh = nc.dram_tensor("osh", [TSH // G, P, G, CB], f32,
                         kind="ExternalOutput")

    # ---- internal DRAM ----
    wxb = nc.dram_tensor("wxb", [T, P, CB], f32, kind="Internal")
    csin = [nc.dram_tensor(f"csin{k}", [ln // G, P, G, CB], f32,
                           kind="Internal")
            for k, ln in enumerate(CHUNKS)]
    csout = [nc.dram_tensor(f"csout{k}", [ln // G // NCORES, P, G, CB], f32,
                            kind="Internal")
             for k, ln in enumerate(CHUNKS)]

    rg = [list(range(NCORES))]

    with tile.TileContext(nc) as tc:
        with tc.tile_pool(name="const", bufs=1) as constp:
            b_sb = constp.tile([P, DC], f32)
            nc.sync.dma_start(b_sb[:], bt[:])
            c_sb = constp.tile([P, 1], f32)
            nc.sync.dma_start(c_sb[:], cst[:])
            zero_bf = constp.tile([P, H], bf16)
            nc.vector.memset(zero_bf[:], 0.0)
            # merged slot weight, transposed: WsT[d_in, d_out] = Ws[d_out, d_in]
            wsT_sb = constp.tile([P, DC, D], bf16)
            wxT_sb = constp.tile([P, DC, D], f32)
            nc.sync.dma_start(wxT_sb[:],
                              wxT.rearrange("(kc p) d -> p kc d", p=P))

            # ============ WsT merge (one-time) ============
            with (
                tc.tile_pool(name="mw", bufs=1) as mwp,
                tc.tile_pool(name="mps", bufs=2, space="PSUM") as mps,
            ):
                v_sb = mwp.tile([P, RC, D], bf16)
                nc.sync.dma_start(v_sb[:],
                                  vnat.rearrange("(rc p) d -> p rc d", p=P))
                uT_sb = mwp.tile([P, RC, D], bf16)
                nc.sync.dma_start(uT_sb[:],
                                  uT.rearrange("(rc p) d -> p rc d", p=P))
                for di in range(DC):
                    for nh in range(2):
                        pm = mps.tile([P, 512], f32)
                        for rc in range(RC):
                            nc.tensor.matmul(
                                pm[:],
                                v_sb[:, rc, di * P:(di + 1) * P],
                                uT_sb[:, rc, nh * 512:(nh + 1) * 512],
                                start=(rc == 0), stop=(rc == RC - 1))
                        nc.scalar.activation(
                            wsT_sb[:, di, nh * 512:(nh + 1) * 512], pm[:],
                            AF.Copy)

            # ============ Wx producer (tile-granular emission) ============
            # state for the incremental emission of Wx matmuls
            xTr = xT.rearrange("(kc p) n -> p kc n", p=P)
            wx_pools = {}
            wxstate = {"k": 0, "mc": 0, "kc": 0, "x_sb": None, "stage": None,
                       "pw": None}

            def wx_emit_mm():
                """Emit one Wx matmul (plus boundary loads/copies/stores)."""
                st = wxstate
                k, mc, kc = st["k"], st["mc"], st["kc"]
                if k >= T // WT:
                    # heater: dummy matmul into scratch psum
                    nc.tensor.matmul(
                        wx_pools["scr"][:], wsT_sb[:, 0, 0:P],
                        wsT_sb[:, 0, 0:512], start=True, stop=True,
                        skip_group_check=True)
                    return
                if mc == 0 and kc == 0:
                    st["x_sb"] = wx_pools["x"].tile([P, DC, WT * B], f32,
                                                    name="wx_x")
                    nc.sync.dma_start(
                        st["x_sb"][:],
                        xTr[:, :, k * WT * B:(k + 1) * WT * B])
                    st["stage"] = wx_pools["st"].tile([P, WT, CB], f32,
                                                      name="wx_stage")
                if kc == 0:
                    st["pw"] = wx_pools["ps"].tile([P, WT * B], f32,
                                                   name="wx_pw")
                nc.tensor.matmul(
                    st["pw"][:],
                    wxT_sb[:, kc, mc * P:(mc + 1) * P],
                    st["x_sb"][:, kc, :],
                    start=(kc == 0), stop=(kc == DC - 1))
                kc += 1
                if kc == DC:
                    kc = 0
                    nc.scalar.activation(
                        st["stage"][:, :, mc * B:(mc + 1) * B],
                        st["pw"].rearrange("p (t b) -> p t b", b=B),
                        AF.Identity, bias=b_sb[:, mc:mc + 1])
                    mc += 1
                    if mc == DC:
                        mc = 0
                        nc.sync.dma_start(
                            wxb[k * WT:(k + 1) * WT].rearrange(
                                "t p c -> p t c"),
                            st["stage"][:])
                        st["k"] = k + 1
                st["mc"], st["kc"] = mc, kc

            with (
                tc.tile_pool(name="wxx", bufs=2) as _xp,
                tc.tile_pool(name="wxs", bufs=2) as _stp,
                tc.tile_pool(name="wxps", bufs=2, space="PSUM") as _wxps,
                tc.tile_pool(name="scrp", bufs=1, space="PSUM") as _scrp,
                tc.tile_pool(name="wl", bufs=3) as wlp,
                tc.tile_pool(name="hs", bufs=3) as hsp,
                tc.tile_pool(name="ps", bufs=3) as psp,
                tc.tile_pool(name="pua", bufs=1, space="PSUM") as puap,
                tc.tile_pool(name="pub", bufs=1, space="PSUM") as pubp,
            ):
                wx_pools["x"] = _xp
                wx_pools["st"] = _stp
                wx_pools["ps"] = _wxps
                wx_pools["scr"] = _scrp.tile([P, 512], f32, name="scr")

                # prelude: first PRE_TILES Wx tiles
                for _ in range(PRE_TILES * DC * DC):
                    wx_emit_mm()

                # psum tiles, statically rotated; bootstrap has_written bits
                pua = [puap.tile([P, H], f32, name=f"pua{i}") for i in range(2)]
                pub = [pubp.tile([P, H], f32, name=f"pub{i}") for i in range(2)]
                for pt in (*pua, *pub):
                    nc.tensor.matmul(pt[:], wsT_sb[:, 0, 0:P], zero_bf[:],
                                     start=True, stop=True)

                hinit = hsp.tile([P, 1, CB], bf16, name="hinit")
                nc.sync.dma_start(hinit[:, 0, :], h0b[:])
                hbf = hinit[:, 0, :]

                wxb_sb = None
                hstage = None
                pstage = None
                ck = 0          # current RS chunk
                ck_off = 0      # its start step
                for t in range(T):
                    if t % G == 0:
                        wxb_sb = wlp.tile([P, G, CB], f32, name="wxl")
                        nc.sync.dma_start(
                            wxb_sb[:],
                            wxb[t:t + G].rearrange("t p c -> p t c"))
                        hstage = hsp.tile([P, G, CB], bf16, name="hstage")
                        pstage = psp.tile([P, G, CB], f32, name="pstage")
                    puA, puB = pua[t % 2], pub[t % 2]
                    ws = wxb_sb[:, t % G, :]
                    # preload Wx+b into PSUM (MMs accumulate onto it)
                    nc.vector.tensor_copy(puA[:], ws[:, 0:H])
                    nc.vector.tensor_copy(puB[:], ws[:, H:CB])
                    newh = hstage[:, t % G, :]
                    # two d_in blocks; within a block d_out-major so each half
                    # of pu completes early in block 2
                    for blk in range(2):
                        for do in range(DC):
                            pu = puA if do < 4 else puB
                            out = pu[:, (do % 4) * B:(do % 4 + 1) * B]
                            for di in range(blk * 4, blk * 4 + 4):
                                nc.tensor.matmul(
                                    out,
                                    wsT_sb[:, di, do * P:(do + 1) * P],
                                    hbf[:, di * B:(di + 1) * B],
                                    start=False,
                                    stop=(blk == 1 and di == blk * 4 + 3),
                                    skip_group_check=True)
                            if blk == 1 and do == 3:
                                nc.scalar.activation(newh[:, 0:H], puA[:],
                                                     AF.Tanh)
                        if blk == 1:
                            nc.scalar.activation(newh[:, H:CB], puB[:],
                                                 AF.Tanh)
                    # PE heater / free Wx precompute in the tanh-wait gap
                    wx_emit_mm()
                    wx_emit_mm()
                    nc.vector.tensor_scalar_mul(pstage[:, t % G, :], newh[:],
                                                c_sb[:])
                    hbf = newh
                    if (t + 1) % G == 0:
                        nc.sync.dma_start(hsh[:, t - G + 1:t + 1, :],
                                          hstage[:])
                        nc.sync.dma_start(
                            csin[ck][(t - G + 1 - ck_off) // G], pstage[:])
                    if t - ck_off + 1 == CHUNKS[ck]:
                        nc.gpsimd.collective_compute(
                            "ReduceScatter", mybir.AluOpType.add,
                            replica_groups=rg,
                            ins=[csin[ck][:]], outs=[csout[ck][:]])
                        ck_off += CHUNKS[ck]
                        ck += 1

            # ============== out = silu(z) * hsum (post-loop) ==============
            with (
                tc.tile_pool(name="cz", bufs=3) as czp,
                tc.tile_pool(name="ch", bufs=3) as chp,
                tc.tile_pool(name="co", bufs=3) as cop,
            ):
                r0 = 0
                for k, ln in enumerate(CHUNKS):
                    ngrp = ln // G // NCORES
                    for g in range(ngrp):
                        hz = chp.tile([P, G, CB], f32, name="hz")
                        nc.sync.dma_start(hz[:], csout[k][g])
                        zz = czp.tile([P, G, CB], f32, name="zz")
                        nc.sync.dma_start(zz[:], zt[r0 + g])
                        sz = cop.tile([P, G, CB], f32, name="sz")
                        nc.scalar.activation(sz[:], zz[:], AF.Silu)
                        oo = cop.tile([P, G, CB], f32, name="oo")
                        nc.vector.tensor_mul(oo[:], sz[:], hz[:])
                        nc.sync.dma_start(osh[r0 + g], oo[:])
                    r0 += ngrp

    nc.compile()
    return nc


_NC_CACHE = {}
_LAST_IN_MAPS = None


def _get_nc():
    if "nc" not in _NC_CACHE:
        _NC_CACHE["nc"] = _build()
    return _NC_CACHE["nc"]


def _pack_bdp(a):
    """[..., B, D] -> [..., P, CB] tile layout: tile[p, dc*B+b] = a[b, dc*P+p]."""
    lead = a.shape[:-2]
    a = a.reshape(lead + (B, DC, P))
    n = len(lead)
    order = tuple(range(n)) + (n + 2, n + 1, n)
    return np.ascontiguousarray(a.transpose(order)).reshape(lead + (P, CB))


def _unpack_bdp(a):
    """[..., P, CB] -> [..., B, D]."""
    lead = a.shape[:-2]
    a = a.reshape(lead + (P, DC, B))
    n = len(lead)
    order = tuple(range(n)) + (n + 2, n + 1, n)
    return np.ascontiguousarray(a.transpose(order)).reshape(lead + (B, D))


def kernel(x, z, h0, W_x, U, V, b, C):
    from concourse.bass_utils import run_bass_kernel_spmd

    x = np.asarray(x, np.float32)
    z = np.asarray(z, np.float32)
    h0 = np.asarray(h0, np.float32)
    W_x = np.asarray(W_x, np.float32)
    U = np.asarray(U, np.float32)
    V = np.asarray(V, np.float32)
    b = np.asarray(b, np.float32)
    C = np.asarray(C, np.float32)

    nc = _get_nc()

    xT_np = np.ascontiguousarray(x.reshape(T * B, D).T)
    wxT_np = np.ascontiguousarray(W_x.T)
    bt_np = np.ascontiguousarray(b.reshape(DC, P).T)

    in_maps = []
    for s in range(NCORES):
        tix = _t_indices(s)
        zp = _pack_bdp(z[tix])                      # [TSH, P, CB]
        zp = np.ascontiguousarray(
            zp.reshape(TSH // G, G, P, CB).transpose(0, 2, 1, 3))
        in_maps.append({
            "xT": xT_np,
            "wxT": wxT_np,
            "vnat": V[s].astype(BF16),
            "uT": np.ascontiguousarray(U[s].T).astype(BF16),
            "bt": bt_np,
            "cst": np.full((P, 1), C[s], np.float32),
            "h0b": _pack_bdp(h0[:, s, :]).astype(BF16),
            "zt": zp,
        })

    global _LAST_IN_MAPS
    _LAST_IN_MAPS = in_maps
    res = run_bass_kernel_spmd(nc, in_maps, core_ids=list(range(NCORES)))

    h = np.empty((T + 1, S, B, D), np.float32)
    out = np.empty((T, B, D), np.float32)
    for s in range(NCORES):
        h[0, s] = h0[:, s, :]
        hp = res.results[s]["hsh"].astype(np.float32)   # [P, T, CB]
        h[1:, s] = _unpack_bdp(np.ascontiguousarray(hp.transpose(1, 0, 2)))
        op = res.results[s]["osh"]                      # [TSH//G, P, G, CB]
        op = np.ascontiguousarray(op.transpose(0, 2, 1, 3)).reshape(
            TSH, P, CB)
        out[_t_indices(s)] = _unpack_bdp(op)
    return out, h
